# revision 15
# baseline (speedup 1.0000x reference)
"""VQ codebook quantization kernel for 8 TRN2 NeuronCores.

Data-parallel: inputs [131072, 64] sharded 16384 rows/core, codebook [512, 64]
replicated. Each core computes, per 128-row tile:
  - A = sum(x^2) per row (f32)
  - m2 = x @ (2*emb)^T   (PE f32 matmul)
  - negd = fl(m2 - fl(A + B))  where B = sum(emb^2) per code
    (This reproduces the reference's fl(fl(A+B) - fl(2m)) rounding, whose
     ulp(~64) quantization creates argmin ties that must be matched.)
  - argmax(negd) with first-index tie-break (max_index)
  - one-hot encodings (is_equal vs iota), quantized rows (indirect DMA gather)
Host combines per-core partials: bincount -> perplexity, sum of min-distances
-> loss, and the straight-through output qst = x + (q - x) in f32.
"""

import numpy as np

import concourse.bass as bass
import concourse.mybir as mybir
from concourse.bacc import Bacc
from concourse.tile import TileContext
from concourse.bass_utils import run_bass_kernel_spmd

N_CORES = 8
N, D, K = 131072, 64, 512
NS = N // N_CORES            # rows per core
COMMITMENT_COST = 0.25

F32 = mybir.dt.float32
U32 = mybir.dt.uint32


def build_nc(ns=NS):
    nt = ns // 128           # tiles per core
    nc = Bacc()
    x_in = nc.declare_dram_parameter("x", [ns, D], F32, isOutput=False)
    e_in = nc.declare_dram_parameter("emb", [K, D], F32, isOutput=False)
    et2_in = nc.declare_dram_parameter("embT2", [D, K], F32, isOutput=False)
    b_in = nc.declare_dram_parameter("bvec", [K], F32, isOutput=False)
    enc_o = nc.declare_dram_parameter("enc", [ns, K], F32, isOutput=True)
    q_o = nc.declare_dram_parameter("q", [ns, D], F32, isOutput=True)
    idx_o = nc.declare_dram_parameter("idx", [128, 8 * nt], U32, isOutput=True)
    sm_o = nc.declare_dram_parameter("sm", [128, 8 * nt], F32, isOutput=True)

    with TileContext(nc) as tc:
        with (
            tc.tile_pool(name="persist", bufs=1) as pp,
            tc.tile_pool(name="psum_t", bufs=2, space="PSUM") as ptp,
            tc.tile_pool(name="psum_m", bufs=2, space="PSUM") as pmp,
            tc.tile_pool(name="xp", bufs=4) as xp,
            tc.tile_pool(name="xtp", bufs=3) as xtp,
            tc.tile_pool(name="abp", bufs=3) as abp,
            tc.tile_pool(name="ndp", bufs=3) as ndp,
            tc.tile_pool(name="encp", bufs=3) as encp,
            tc.tile_pool(name="qp", bufs=3) as qp,
            tc.tile_pool(name="smallp", bufs=4) as smp,
        ):
            # ---------------- persistent tiles ----------------
            ident = pp.tile([128, 128], F32)        # identity for PE transpose
            embT2 = pp.tile([64, K], F32)           # (2*emb)^T
            b_full = pp.tile([128, K], F32)         # B broadcast to 128 parts
            iota512 = pp.tile([128, K], F32)        # 0..511 each row
            a_all = pp.tile([128, nt], F32)         # per-tile row sumsq
            smax_all = pp.tile([128, 8 * nt], F32)  # max8 outputs per tile
            idx_all = pp.tile([128, 8 * nt], U32)   # max_index outputs

            # ---------------- setup ----------------
            # identity matrix via f32 iota + is_equal
            iota_row = pp.tile([128, 128], F32)
            iota_col = pp.tile([128, 1], F32)
            nc.gpsimd.iota(iota_row[:, :], pattern=[[1, 128]], base=0,
                           channel_multiplier=0,
                           allow_small_or_imprecise_dtypes=True)
            nc.gpsimd.iota(iota_col[:, :], pattern=[[1, 1]], base=0,
                           channel_multiplier=1,
                           allow_small_or_imprecise_dtypes=True)
            nc.vector.tensor_scalar(ident[:, :], iota_row[:, :],
                                    iota_col[:, 0:1], None,
                                    op0=mybir.AluOpType.is_equal)
            nc.gpsimd.iota(iota512[:, :], pattern=[[1, K]], base=0,
                           channel_multiplier=0,
                           allow_small_or_imprecise_dtypes=True)

            nc.sync.dma_start(embT2[:, :], et2_in[:, :])
            nc.sync.dma_start(b_full[:, :], b_in[None, :].to_broadcast([128, K]))

            # ---------------- main loop ----------------
            for t in range(nt):
                x_t = xp.tile([128, D], F32)
                nc.sync.dma_start(x_t[:, :], x_in[bass.ts(t, 128), :])

                # A = sum(x^2) per row (accum_out); main out is scratch
                sc = smp.tile([128, D], F32, tag="sq_scratch")
                nc.scalar.activation(
                    out=sc[:, :], in_=x_t[:, :],
                    func=mybir.ActivationFunctionType.Square,
                    accum_out=a_all[:, t:t + 1])

                # xT via PE transpose, then PSUM->SBUF copy
                xt_ps = ptp.tile([64, 128], F32, tag="xt")
                nc.tensor.transpose(out=xt_ps[:, :], in_=x_t[:, :],
                                    identity=ident[:, :])
                xt = xtp.tile([64, 128], F32)
                nc.vector.tensor_copy(xt[:, :], xt_ps[:, :])

                # m2 = x @ (2 emb)^T  [128, 512] PSUM
                m2 = pmp.tile([128, K], F32)
                nc.tensor.matmul(out=m2[:, :], lhsT=xt[:, :], rhs=embT2[:, :],
                                 start=True, stop=True)

                # AB = fl(A + B) on ACT
                ab = abp.tile([128, K], F32)
                nc.scalar.activation(
                    out=ab[:, :], in_=b_full[:, :],
                    func=mybir.ActivationFunctionType.Identity,
                    bias=a_all[:, t:t + 1], scale=1.0)

                # negd = fl(m2*1.0 - AB)  (one pass; PSUM read)
                nd = ndp.tile([128, K], F32)
                nc.vector.scalar_tensor_tensor(
                    out=nd[:, :], in0=m2[:, :], scalar=1.0, in1=ab[:, :],
                    op0=mybir.AluOpType.mult, op1=mybir.AluOpType.subtract)

                # argmax with first-index tie-break; max8 out doubles as
                # the loss partial (col 8t holds the row max)
                nc.vector.max(out=smax_all[:, 8 * t:8 * t + 8], in_=nd[:, :])
                nc.vector.max_index(out=idx_all[:, 8 * t:8 * t + 8],
                                    in_max=smax_all[:, 8 * t:8 * t + 8],
                                    in_values=nd[:, :])

                # one-hot on gpsimd: is_equal(iota, idx)
                idxf = smp.tile([128, 1], F32, tag="idxf")
                nc.vector.tensor_copy(idxf[:, :], idx_all[:, 8 * t:8 * t + 1])
                enc_t = encp.tile([128, K], F32)
                nc.gpsimd.tensor_scalar(enc_t[:, :], iota512[:, :],
                                        idxf[:, 0:1], None,
                                        op0=mybir.AluOpType.is_equal)
                nc.sync.dma_start(enc_o[bass.ts(t, 128), :], enc_t[:, :])

                # gather quantized rows: q[p] = emb[idx[p]]
                q_t = qp.tile([128, D], F32)
                nc.gpsimd.indirect_dma_start(
                    out=q_t[:, :], out_offset=None, in_=e_in[:, :],
                    in_offset=bass.IndirectOffsetOnAxis(
                        ap=idx_all[:, 8 * t:8 * t + 1], axis=0))
                nc.sync.dma_start(q_o[bass.ts(t, 128), :], q_t[:, :])

            # ---------------- tail ----------------
            nc.sync.dma_start(sm_o[:, :], smax_all[:, :])
            nc.sync.dma_start(idx_o[:, :], idx_all[:, :])

    if not nc.is_finalized():
        nc.finalize()
    return nc


_NC_CACHE = {}


def _get_nc(ns=NS):
    if ns not in _NC_CACHE:
        _NC_CACHE[ns] = build_nc(ns)
    return _NC_CACHE[ns]


def kernel(inputs: np.ndarray, emb_w: np.ndarray, _trace=False):
    inputs = np.ascontiguousarray(np.asarray(inputs, dtype=np.float32))
    emb_w = np.ascontiguousarray(np.asarray(emb_w, dtype=np.float32))
    assert inputs.shape == (N, D) and emb_w.shape == (K, D)

    nc = _get_nc()
    embT2 = np.ascontiguousarray((np.float32(2.0) * emb_w).T)
    bvec = np.sum(emb_w.astype(np.float32) ** 2, axis=1,
                  dtype=np.float32).astype(np.float32)
    in_maps = [
        {"x": inputs[i * NS:(i + 1) * NS], "emb": emb_w,
         "embT2": embT2, "bvec": bvec}
        for i in range(N_CORES)
    ]
    out = run_bass_kernel_spmd(nc, in_maps, core_ids=list(range(N_CORES)),
                               trace=_trace)
    res = out.results

    enc = np.concatenate([r["enc"] for r in res], axis=0)
    q = np.concatenate([r["q"] for r in res], axis=0)
    nt = NS // 128
    idx = np.concatenate([
        r["idx"][:, 0::8].T.reshape(-1) for r in res
    ])  # [p, t] -> row t*128+p order

    # loss: sum over rows of min distance == -sum(smax)
    s_smax = sum(float(r["sm"][:, 0::8].sum(dtype=np.float64)) for r in res)
    mse = (-s_smax) / (N * D)
    loss = np.float32((1.0 + COMMITMENT_COST) * mse)

    counts = np.bincount(idx, minlength=K).astype(np.float64)
    avg_probs = counts / N
    perplexity = np.float32(np.exp(-np.sum(avg_probs * np.log(avg_probs + 1e-10))))

    # straight-through output, f32 exact as reference computes it
    quantized_st = inputs + (q - inputs)

    if _trace:
        return (loss, quantized_st, perplexity, enc), out
    return loss, quantized_st, perplexity, enc


# revision 22
# speedup vs baseline: 3.0665x; 3.0665x over previous
"""VQ codebook quantization kernel for 8 TRN2 NeuronCores.

Data-parallel: inputs [131072, 64] sharded 16384 rows/core, codebook [512, 64]
replicated. Each core computes, per 128-row tile:
  - A = sum(x^2) per row (f32)
  - m2 = x @ (2*emb)^T   (PE f32 matmul)
  - negd = fl(m2 - fl(A + B))  where B = sum(emb^2) per code
    (This reproduces the reference's fl(fl(A+B) - fl(2m)) rounding, whose
     ulp(~64) quantization creates argmin ties that must be matched.)
  - argmax(negd) with first-index tie-break (max_index)
  - one-hot encodings (is_equal vs iota), quantized rows (indirect DMA gather)
Host combines per-core partials: bincount -> perplexity, sum of min-distances
-> loss, and the straight-through output qst = x + (q - x) in f32.
"""

import numpy as np

import concourse.bass as bass
import concourse.mybir as mybir
from concourse.bacc import Bacc
from concourse.tile import TileContext
from concourse.bass_utils import run_bass_kernel_spmd

N_CORES = 8
N, D, K = 131072, 64, 512
NS = N // N_CORES            # rows per core
COMMITMENT_COST = 0.25

F32 = mybir.dt.float32
U32 = mybir.dt.uint32


def build_nc(ns=NS):
    nt = ns // 128           # tiles per core
    nc = Bacc()
    x_in = nc.declare_dram_parameter("x", [ns, D], F32, isOutput=False)
    e_in = nc.declare_dram_parameter("emb", [K, D], F32, isOutput=False)
    et2_in = nc.declare_dram_parameter("embT2", [D, K], F32, isOutput=False)
    b_in = nc.declare_dram_parameter("bvec", [K], F32, isOutput=False)
    enc_o = nc.declare_dram_parameter("enc", [ns, K], F32, isOutput=True)
    q_o = nc.declare_dram_parameter("q", [ns, D], F32, isOutput=True)
    idx_o = nc.declare_dram_parameter("idx", [128, 8 * nt], U32, isOutput=True)
    sm_o = nc.declare_dram_parameter("sm", [128, 8 * nt], F32, isOutput=True)

    with TileContext(nc) as tc:
        with (
            tc.tile_pool(name="persist", bufs=1) as pp,
            tc.tile_pool(name="psum_t", bufs=2, space="PSUM") as ptp,
            tc.tile_pool(name="psum_m", bufs=2, space="PSUM") as pmp,
            tc.tile_pool(name="xp", bufs=4) as xp,
            tc.tile_pool(name="xtp", bufs=3) as xtp,
            tc.tile_pool(name="abp", bufs=3) as abp,
            tc.tile_pool(name="ndp", bufs=3) as ndp,
            tc.tile_pool(name="encp", bufs=3) as encp,
            tc.tile_pool(name="qp", bufs=3) as qp,
            tc.tile_pool(name="smallp", bufs=4) as smp,
        ):
            # ---------------- persistent tiles ----------------
            ident = pp.tile([128, 128], F32)        # identity for PE transpose
            embT2 = pp.tile([64, K], F32)           # (2*emb)^T
            b_full = pp.tile([128, K], F32)         # B broadcast to 128 parts
            iota512 = pp.tile([128, K], F32)        # 0..511 each row
            a_all = pp.tile([128, nt], F32)         # per-tile row sumsq
            smax_all = pp.tile([128, 8 * nt], F32)  # max8 outputs per tile
            idx_all = pp.tile([128, 8 * nt], U32)   # max_index outputs

            # ---------------- setup ----------------
            # identity matrix via f32 iota + is_equal
            iota_row = pp.tile([128, 128], F32)
            iota_col = pp.tile([128, 1], F32)
            nc.gpsimd.iota(iota_row[:, :], pattern=[[1, 128]], base=0,
                           channel_multiplier=0,
                           allow_small_or_imprecise_dtypes=True)
            nc.gpsimd.iota(iota_col[:, :], pattern=[[1, 1]], base=0,
                           channel_multiplier=1,
                           allow_small_or_imprecise_dtypes=True)
            nc.vector.tensor_scalar(ident[:, :], iota_row[:, :],
                                    iota_col[:, 0:1], None,
                                    op0=mybir.AluOpType.is_equal)
            nc.gpsimd.iota(iota512[:, :], pattern=[[1, K]], base=0,
                           channel_multiplier=0,
                           allow_small_or_imprecise_dtypes=True)

            nc.sync.dma_start(embT2[:, :], et2_in[:, :])
            nc.sync.dma_start(b_full[:, :], b_in[None, :].to_broadcast([128, K]))

            # ---------------- main loop ----------------
            for t in range(nt):
                x_t = xp.tile([128, D], F32)
                nc.sync.dma_start(x_t[:, :], x_in[bass.ts(t, 128), :])

                # A = sum(x^2) per row (accum_out); main out is scratch
                sc = smp.tile([128, D], F32, tag="sq_scratch")
                nc.scalar.activation(
                    out=sc[:, :], in_=x_t[:, :],
                    func=mybir.ActivationFunctionType.Square,
                    accum_out=a_all[:, t:t + 1])

                # xT via PE transpose, then PSUM->SBUF copy (on ACT)
                xt_ps = ptp.tile([64, 128], F32, tag="xt")
                nc.tensor.transpose(out=xt_ps[:, :], in_=x_t[:, :],
                                    identity=ident[:, :])
                xt = xtp.tile([64, 128], F32)
                nc.scalar.activation(out=xt[:, :], in_=xt_ps[:, :],
                                     func=mybir.ActivationFunctionType.Copy)

                # m2 = x @ (2 emb)^T  [128, 512] PSUM
                m2 = pmp.tile([128, K], F32)
                nc.tensor.matmul(out=m2[:, :], lhsT=xt[:, :], rhs=embT2[:, :],
                                 start=True, stop=True)

                # AB = fl(A + B) on ACT
                ab = abp.tile([128, K], F32)
                nc.scalar.activation(
                    out=ab[:, :], in_=b_full[:, :],
                    func=mybir.ActivationFunctionType.Identity,
                    bias=a_all[:, t:t + 1], scale=1.0)

                # negd = fl(m2*1.0 - AB)  (one pass; PSUM read)
                nd = ndp.tile([128, K], F32)
                nc.vector.scalar_tensor_tensor(
                    out=nd[:, :], in0=m2[:, :], scalar=1.0, in1=ab[:, :],
                    op0=mybir.AluOpType.mult, op1=mybir.AluOpType.subtract)

                # argmax with first-index tie-break; max8 out doubles as
                # the loss partial (col 8t holds the row max)
                nc.vector.max(out=smax_all[:, 8 * t:8 * t + 8], in_=nd[:, :])
                nc.vector.max_index(out=idx_all[:, 8 * t:8 * t + 8],
                                    in_max=smax_all[:, 8 * t:8 * t + 8],
                                    in_values=nd[:, :])

                # one-hot on DVE: is_equal(f32 iota, f32 idx) -> f32
                idxf = smp.tile([128, 1], F32, tag="idxf")
                nc.vector.tensor_copy(idxf[:, :], idx_all[:, 8 * t:8 * t + 1])
                enc_t = encp.tile([128, K], F32)
                nc.vector.tensor_scalar(enc_t[:, :], iota512[:, :],
                                        idxf[:, 0:1], None,
                                        op0=mybir.AluOpType.is_equal)
                nc.sync.dma_start(enc_o[bass.ts(t, 128), :], enc_t[:, :])

                # gather quantized rows: q[p] = emb[idx[p]]
                q_t = qp.tile([128, D], F32)
                nc.gpsimd.indirect_dma_start(
                    out=q_t[:, :], out_offset=None, in_=e_in[:, :],
                    in_offset=bass.IndirectOffsetOnAxis(
                        ap=idx_all[:, 8 * t:8 * t + 1], axis=0))
                nc.sync.dma_start(q_o[bass.ts(t, 128), :], q_t[:, :])

            # ---------------- tail ----------------
            nc.sync.dma_start(sm_o[:, :], smax_all[:, :])
            nc.sync.dma_start(idx_o[:, :], idx_all[:, :])

    if not nc.is_finalized():
        nc.finalize()
    return nc


_NC_CACHE = {}


def _get_nc(ns=NS):
    if ns not in _NC_CACHE:
        _NC_CACHE[ns] = build_nc(ns)
    return _NC_CACHE[ns]


def kernel(inputs: np.ndarray, emb_w: np.ndarray, _trace=False):
    inputs = np.ascontiguousarray(np.asarray(inputs, dtype=np.float32))
    emb_w = np.ascontiguousarray(np.asarray(emb_w, dtype=np.float32))
    assert inputs.shape == (N, D) and emb_w.shape == (K, D)

    nc = _get_nc()
    embT2 = np.ascontiguousarray((np.float32(2.0) * emb_w).T)
    bvec = np.sum(emb_w.astype(np.float32) ** 2, axis=1,
                  dtype=np.float32).astype(np.float32)
    in_maps = [
        {"x": inputs[i * NS:(i + 1) * NS], "emb": emb_w,
         "embT2": embT2, "bvec": bvec}
        for i in range(N_CORES)
    ]
    out = run_bass_kernel_spmd(nc, in_maps, core_ids=list(range(N_CORES)),
                               trace=_trace)
    res = out.results

    enc = np.concatenate([r["enc"] for r in res], axis=0)
    q = np.concatenate([r["q"] for r in res], axis=0)
    nt = NS // 128
    idx = np.concatenate([
        r["idx"][:, 0::8].T.reshape(-1) for r in res
    ])  # [p, t] -> row t*128+p order

    # loss: sum over rows of min distance == -sum(smax)
    s_smax = sum(float(r["sm"][:, 0::8].sum(dtype=np.float64)) for r in res)
    mse = (-s_smax) / (N * D)
    loss = np.float32((1.0 + COMMITMENT_COST) * mse)

    counts = np.bincount(idx, minlength=K).astype(np.float64)
    avg_probs = counts / N
    perplexity = np.float32(np.exp(-np.sum(avg_probs * np.log(avg_probs + 1e-10))))

    # straight-through output, f32 exact as reference computes it
    quantized_st = inputs + (q - inputs)

    if _trace:
        return (loss, quantized_st, perplexity, enc), out
    return loss, quantized_st, perplexity, enc


# revision 23
# speedup vs baseline: 3.2089x; 1.0464x over previous
"""VQ codebook quantization kernel for 8 TRN2 NeuronCores.

Data-parallel: inputs [131072, 64] sharded 16384 rows/core, codebook [512, 64]
replicated. Per 128-row tile the device computes:
  - s = x @ (2*emb)^T - B   via one PE matmul with an augmented contraction
    row (lhsT row 64 = ones, rhs row 64 = -B), where B = sum(emb^2) per code
  - argmax(s) (max8 + max_index), one-hot encodings, quantized rows (indirect
    DMA gather), and per-row sum(x^2) for the loss.
The reference computes distances in f32 at magnitude ||x||^2 ~ 64, which
quantizes them to a ~7.6e-6 grid and creates near-ties; the device ranks by
the fine-grained s instead, and the host repairs the ~0.5% of rows whose
top-2 gap is below a threshold by recomputing them with the reference's exact
f32 rounding semantics.
"""

import numpy as np

import concourse.bass as bass
import concourse.mybir as mybir
from concourse.bacc import Bacc
from concourse.tile import TileContext
from concourse.bass_utils import run_bass_kernel_spmd

N_CORES = 8
N, D, K = 131072, 64, 512
NS = N // N_CORES            # rows per core
COMMITMENT_COST = 0.25
GAP_THRESHOLD = 5e-5         # flag rows for host repair

F32 = mybir.dt.float32
U32 = mybir.dt.uint32


def build_nc(ns=NS):
    nt = ns // 128           # tiles per core
    nc = Bacc()
    x_in = nc.declare_dram_parameter("x", [ns, D], F32, isOutput=False)
    e_in = nc.declare_dram_parameter("emb", [K, D], F32, isOutput=False)
    et2_in = nc.declare_dram_parameter("embT2B", [D + 1, K], F32,
                                       isOutput=False)
    enc_o = nc.declare_dram_parameter("enc", [ns, K], F32, isOutput=True)
    q_o = nc.declare_dram_parameter("q", [ns, D], F32, isOutput=True)
    idx_o = nc.declare_dram_parameter("idx", [128, 8 * nt], U32, isOutput=True)
    sm_o = nc.declare_dram_parameter("sm", [128, 8 * nt], F32, isOutput=True)
    av_o = nc.declare_dram_parameter("av", [128, nt], F32, isOutput=True)

    with TileContext(nc) as tc:
        with (
            tc.tile_pool(name="persist", bufs=1) as pp,
            tc.tile_pool(name="psum_t", bufs=2, space="PSUM") as ptp,
            tc.tile_pool(name="psum_m", bufs=2, space="PSUM") as pmp,
            tc.tile_pool(name="xp", bufs=4) as xp,
            tc.tile_pool(name="xtp", bufs=3) as xtp,
            tc.tile_pool(name="sp", bufs=3) as sp,
            tc.tile_pool(name="encp", bufs=3) as encp,
            tc.tile_pool(name="qp", bufs=3) as qp,
            tc.tile_pool(name="smallp", bufs=4) as smp,
        ):
            # ---------------- persistent tiles ----------------
            ident = pp.tile([128, 128], F32)        # identity for PE transpose
            embT2B = pp.tile([D + 1, K], F32)       # rows 0-63: (2emb)^T; 64: -B
            iota512 = pp.tile([128, K], F32)        # 0..511 each row
            a_all = pp.tile([128, nt], F32)         # per-tile row sumsq
            smax_all = pp.tile([128, 8 * nt], F32)  # max8 outputs per tile
            idx_all = pp.tile([128, 8 * nt], U32)   # max_index outputs

            # ---------------- setup ----------------
            iota_row = pp.tile([128, 128], F32)
            iota_col = pp.tile([128, 1], F32)
            nc.gpsimd.iota(iota_row[:, :], pattern=[[1, 128]], base=0,
                           channel_multiplier=0,
                           allow_small_or_imprecise_dtypes=True)
            nc.gpsimd.iota(iota_col[:, :], pattern=[[1, 1]], base=0,
                           channel_multiplier=1,
                           allow_small_or_imprecise_dtypes=True)
            nc.vector.tensor_scalar(ident[:, :], iota_row[:, :],
                                    iota_col[:, 0:1], None,
                                    op0=mybir.AluOpType.is_equal)
            nc.gpsimd.iota(iota512[:, :], pattern=[[1, K]], base=0,
                           channel_multiplier=0,
                           allow_small_or_imprecise_dtypes=True)
            nc.sync.dma_start(embT2B[:, :], et2_in[:, :])

            # ---------------- main loop ----------------
            for t in range(nt):
                x_t = xp.tile([128, D], F32)
                nc.sync.dma_start(x_t[:, :], x_in[bass.ts(t, 128), :])

                # A = sum(x^2) per row (accum_out); main out is scratch
                sc = smp.tile([128, D], F32, tag="sq_scratch")
                nc.scalar.activation(
                    out=sc[:, :], in_=x_t[:, :],
                    func=mybir.ActivationFunctionType.Square,
                    accum_out=a_all[:, t:t + 1])

                # xT via PE transpose; augmented lhsT row 64 = ones
                xt_ps = ptp.tile([64, 128], F32, tag="xt")
                nc.tensor.transpose(out=xt_ps[:, :], in_=x_t[:, :],
                                    identity=ident[:, :])
                xt = xtp.tile([D + 1, 128], F32)
                nc.scalar.activation(out=xt[0:64, :], in_=xt_ps[:, :],
                                     func=mybir.ActivationFunctionType.Copy)
                nc.vector.memset(xt[64:65, :], 1.0)

                # s = x @ (2 emb)^T - B   [128, 512] PSUM
                m2 = pmp.tile([128, K], F32)
                nc.tensor.matmul(out=m2[:, :], lhsT=xt[:, :],
                                 rhs=embT2B[:, :], start=True, stop=True)

                # s PSUM -> SBUF (ACT)
                s_sb = sp.tile([128, K], F32)
                nc.scalar.activation(out=s_sb[:, :], in_=m2[:, :],
                                     func=mybir.ActivationFunctionType.Copy)

                # argmax; max8 out doubles as the loss/gap output
                nc.vector.max(out=smax_all[:, 8 * t:8 * t + 8], in_=s_sb[:, :])
                nc.vector.max_index(out=idx_all[:, 8 * t:8 * t + 8],
                                    in_max=smax_all[:, 8 * t:8 * t + 8],
                                    in_values=s_sb[:, :])

                # one-hot on DVE: is_equal(f32 iota, f32 idx)
                idxf = smp.tile([128, 1], F32, tag="idxf")
                nc.vector.tensor_copy(idxf[:, :], idx_all[:, 8 * t:8 * t + 1])
                enc_t = encp.tile([128, K], F32)
                nc.vector.tensor_scalar(enc_t[:, :], iota512[:, :],
                                        idxf[:, 0:1], None,
                                        op0=mybir.AluOpType.is_equal)
                nc.sync.dma_start(enc_o[bass.ts(t, 128), :], enc_t[:, :])

                # gather quantized rows: q[p] = emb[idx[p]]
                q_t = qp.tile([128, D], F32)
                nc.gpsimd.indirect_dma_start(
                    out=q_t[:, :], out_offset=None, in_=e_in[:, :],
                    in_offset=bass.IndirectOffsetOnAxis(
                        ap=idx_all[:, 8 * t:8 * t + 1], axis=0))
                nc.sync.dma_start(q_o[bass.ts(t, 128), :], q_t[:, :])

            # ---------------- tail ----------------
            nc.sync.dma_start(sm_o[:, :], smax_all[:, :])
            nc.sync.dma_start(idx_o[:, :], idx_all[:, :])
            nc.sync.dma_start(av_o[:, :], a_all[:, :])

    if not nc.is_finalized():
        nc.finalize()
    return nc


_NC_CACHE = {}


def _get_nc(ns=NS):
    if ns not in _NC_CACHE:
        _NC_CACHE[ns] = build_nc(ns)
    return _NC_CACHE[ns]


def _host_prep(emb_w):
    embT2B = np.empty((D + 1, K), np.float32)
    embT2B[:D] = (np.float32(2.0) * emb_w).T
    bvec = np.sum(emb_w.astype(np.float32) ** 2, axis=1,
                  dtype=np.float32).astype(np.float32)
    embT2B[D] = -bvec
    return np.ascontiguousarray(embT2B), bvec


def kernel(inputs: np.ndarray, emb_w: np.ndarray, _trace=False):
    inputs = np.ascontiguousarray(np.asarray(inputs, dtype=np.float32))
    emb_w = np.ascontiguousarray(np.asarray(emb_w, dtype=np.float32))
    assert inputs.shape == (N, D) and emb_w.shape == (K, D)

    nc = _get_nc()
    embT2B, bvec = _host_prep(emb_w)
    in_maps = [
        {"x": inputs[i * NS:(i + 1) * NS], "emb": emb_w, "embT2B": embT2B}
        for i in range(N_CORES)
    ]
    out = run_bass_kernel_spmd(nc, in_maps, core_ids=list(range(N_CORES)),
                               trace=_trace)
    res = out.results

    enc = np.concatenate([r["enc"] for r in res], axis=0)
    q = np.concatenate([r["q"] for r in res], axis=0)
    nt = NS // 128
    # device row r of core i lives at [p=r%128, t=r//128]
    idx = np.concatenate([r["idx"][:, 0::8].T.reshape(-1) for r in res])
    smax0 = np.concatenate([r["sm"][:, 0::8].T.reshape(-1) for r in res])
    smax1 = np.concatenate([r["sm"][:, 1::8].T.reshape(-1) for r in res])
    avals = np.concatenate([r["av"].T.reshape(-1) for r in res])

    # per-row min distance for the loss (fine-grained, unquantized)
    dmin = avals.astype(np.float64) - smax0.astype(np.float64)

    # ---- host repair of near-tie rows: reproduce the reference's f32
    # rounding (distances at magnitude ~64 quantize to a ~7.6e-6 grid)
    flagged = np.flatnonzero((smax0 - smax1) < GAP_THRESHOLD)
    if flagged.size:
        xr = inputs[flagged]
        a32 = np.sum(xr ** 2, axis=1, dtype=np.float32).astype(np.float32)
        m32 = xr @ emb_w.T
        d32 = ((a32[:, None] + bvec[None, :]).astype(np.float32)
               - np.float32(2.0) * m32).astype(np.float32)
        idx_fix = np.argmin(d32, axis=1)
        changed = idx_fix != idx[flagged]
        rows = flagged[changed]
        if rows.size:
            idx[rows] = idx_fix[changed]
            enc[rows] = 0.0
            enc[rows, idx[rows]] = 1.0
            q[rows] = emb_w[idx[rows]]
        dmin[flagged] = d32.min(axis=1).astype(np.float64)

    loss = np.float32((1.0 + COMMITMENT_COST) * (dmin.sum() / (N * D)))

    counts = np.bincount(idx, minlength=K).astype(np.float64)
    avg_probs = counts / N
    perplexity = np.float32(
        np.exp(-np.sum(avg_probs * np.log(avg_probs + 1e-10))))

    # straight-through output, f32 exact as reference computes it
    quantized_st = inputs + (q - inputs)

    if _trace:
        return (loss, quantized_st, perplexity, enc), out
    return loss, quantized_st, perplexity, enc


# revision 28
# speedup vs baseline: 3.8141x; 1.1886x over previous
"""VQ codebook quantization kernel for 8 TRN2 NeuronCores.

Data-parallel: inputs [131072, 64] sharded 16384 rows/core, codebook [512, 64]
replicated. Per 128-row tile the device computes:
  - s = x @ (2*emb)^T - B   via one PE matmul with an augmented contraction
    row (lhsT row 64 = ones, rhs row 64 = -B), where B = sum(emb^2) per code
  - argmax(s) (max8 + max_index), one-hot encodings, quantized rows (indirect
    DMA gather), and per-row sum(x^2) for the loss.
The reference computes distances in f32 at magnitude ||x||^2 ~ 64, which
quantizes them to a ~7.6e-6 grid and creates near-ties; the device ranks by
the fine-grained s instead, and the host repairs the ~0.5% of rows whose
top-2 gap is below a threshold by recomputing them with the reference's exact
f32 rounding semantics.
"""

import numpy as np

import concourse.bass as bass
import concourse.mybir as mybir
from concourse.bacc import Bacc
from concourse.tile import TileContext
from concourse.bass_utils import run_bass_kernel_spmd

N_CORES = 8
N, D, K = 131072, 64, 512
NS = N // N_CORES            # rows per core
COMMITMENT_COST = 0.25
GAP_THRESHOLD = 5e-5         # flag rows for host repair

F32 = mybir.dt.float32
F32R = mybir.dt.float32r
U32 = mybir.dt.uint32


def build_nc(ns=NS):
    nt = ns // 128           # tiles per core
    nc = Bacc()
    x_in = nc.declare_dram_parameter("x", [ns, D], F32, isOutput=False)
    e_in = nc.declare_dram_parameter("emb", [K, D], F32, isOutput=False)
    et2_in = nc.declare_dram_parameter("embT2B", [D + 1, K], F32,
                                       isOutput=False)
    enc_o = nc.declare_dram_parameter("enc", [ns, K], F32, isOutput=True)
    q_o = nc.declare_dram_parameter("q", [ns, D], F32, isOutput=True)
    idx_o = nc.declare_dram_parameter("idx", [128, 8 * nt], U32, isOutput=True)
    sm_o = nc.declare_dram_parameter("sm", [128, 8 * nt], F32, isOutput=True)
    av_o = nc.declare_dram_parameter("av", [128, nt], F32, isOutput=True)

    with TileContext(nc) as tc:
        with (
            tc.tile_pool(name="persist", bufs=1) as pp,
            tc.tile_pool(name="psum_t", bufs=2, space="PSUM") as ptp,
            tc.tile_pool(name="psum_m", bufs=2, space="PSUM") as pmp,
            tc.tile_pool(name="xp", bufs=4) as xp,
            tc.tile_pool(name="xtp", bufs=3) as xtp,
            tc.tile_pool(name="sp", bufs=3) as sp,
            tc.tile_pool(name="encp", bufs=3) as encp,
            tc.tile_pool(name="qp", bufs=3) as qp,
            tc.tile_pool(name="smallp", bufs=4) as smp,
        ):
            # ---------------- persistent tiles ----------------
            ident = pp.tile([128, 128], F32)        # identity for PE transpose
            embT2B = pp.tile([D + 1, K], F32)       # rows 0-63: (2emb)^T; 64: -B
            embT2Br = pp.tile([D + 1, K], F32R)     # f32r-rounded copy
            iota512 = pp.tile([128, K], F32)        # 0..511 each row
            a_all = pp.tile([128, nt], F32)         # per-tile row sumsq
            smax_all = pp.tile([128, 8 * nt], F32)  # max8 outputs per tile
            idx_all = pp.tile([128, 8 * nt], U32)   # max_index outputs

            # ---------------- setup ----------------
            iota_row = pp.tile([128, 128], F32)
            iota_col = pp.tile([128, 1], F32)
            nc.gpsimd.iota(iota_row[:, :], pattern=[[1, 128]], base=0,
                           channel_multiplier=0,
                           allow_small_or_imprecise_dtypes=True)
            nc.gpsimd.iota(iota_col[:, :], pattern=[[1, 1]], base=0,
                           channel_multiplier=1,
                           allow_small_or_imprecise_dtypes=True)
            nc.vector.tensor_scalar(ident[:, :], iota_row[:, :],
                                    iota_col[:, 0:1], None,
                                    op0=mybir.AluOpType.is_equal)
            nc.gpsimd.iota(iota512[:, :], pattern=[[1, K]], base=0,
                           channel_multiplier=0,
                           allow_small_or_imprecise_dtypes=True)
            nc.sync.dma_start(embT2B[:, :], et2_in[:, :])
            nc.vector.tensor_copy(embT2Br[:, :], embT2B[:, :])

            # ---------------- main loop ----------------
            for t in range(nt):
                x_t = xp.tile([128, D], F32)
                nc.sync.dma_start(x_t[:, :], x_in[bass.ts(t, 128), :])

                # A = sum(x^2) per row (accum_out); main out is scratch
                sc = smp.tile([128, D], F32, tag="sq_scratch")
                nc.scalar.activation(
                    out=sc[:, :], in_=x_t[:, :],
                    func=mybir.ActivationFunctionType.Square,
                    accum_out=a_all[:, t:t + 1])

                # xT via PE transpose; augmented lhsT row 64 = ones
                xt_ps = ptp.tile([64, 128], F32, tag="xt")
                nc.tensor.transpose(out=xt_ps[:, :], in_=x_t[:, :],
                                    identity=ident[:, :])
                xt = xtp.tile([D + 1, 128], F32R)
                nc.scalar.activation(out=xt[0:64, :], in_=xt_ps[:, :],
                                     func=mybir.ActivationFunctionType.Copy)
                nc.vector.memset(xt[64:65, :].bitcast(F32), 1.0)

                # s = x @ (2 emb)^T - B   [128, 512] PSUM (f32r matmul)
                m2 = pmp.tile([128, K], F32)
                nc.tensor.matmul(out=m2[:, :], lhsT=xt[:, :],
                                 rhs=embT2Br[:, :], start=True, stop=True)

                # s PSUM -> SBUF (ACT)
                s_sb = sp.tile([128, K], F32)
                nc.scalar.activation(out=s_sb[:, :], in_=m2[:, :],
                                     func=mybir.ActivationFunctionType.Copy)

                # argmax; max8 out doubles as the loss/gap output
                nc.vector.max(out=smax_all[:, 8 * t:8 * t + 8], in_=s_sb[:, :])
                nc.vector.max_index(out=idx_all[:, 8 * t:8 * t + 8],
                                    in_max=smax_all[:, 8 * t:8 * t + 8],
                                    in_values=s_sb[:, :])

                # one-hot on DVE: is_equal(f32 iota, f32 idx)
                idxf = smp.tile([128, 1], F32, tag="idxf")
                nc.vector.tensor_copy(idxf[:, :], idx_all[:, 8 * t:8 * t + 1])
                enc_t = encp.tile([128, K], F32)
                nc.vector.tensor_scalar(enc_t[:, :], iota512[:, :],
                                        idxf[:, 0:1], None,
                                        op0=mybir.AluOpType.is_equal)
                nc.sync.dma_start(enc_o[bass.ts(t, 128), :], enc_t[:, :])

                # gather quantized rows: q[p] = emb[idx[p]]
                q_t = qp.tile([128, D], F32)
                nc.gpsimd.indirect_dma_start(
                    out=q_t[:, :], out_offset=None, in_=e_in[:, :],
                    in_offset=bass.IndirectOffsetOnAxis(
                        ap=idx_all[:, 8 * t:8 * t + 1], axis=0))
                nc.sync.dma_start(q_o[bass.ts(t, 128), :], q_t[:, :])

            # ---------------- tail ----------------
            nc.sync.dma_start(sm_o[:, :], smax_all[:, :])
            nc.sync.dma_start(idx_o[:, :], idx_all[:, :])
            nc.sync.dma_start(av_o[:, :], a_all[:, :])

    if not nc.is_finalized():
        nc.finalize()
    return nc


_NC_CACHE = {}


def _get_nc(ns=NS):
    if ns not in _NC_CACHE:
        _NC_CACHE[ns] = build_nc(ns)
    return _NC_CACHE[ns]


def _host_prep(emb_w):
    embT2B = np.empty((D + 1, K), np.float32)
    embT2B[:D] = (np.float32(2.0) * emb_w).T
    bvec = np.sum(emb_w.astype(np.float32) ** 2, axis=1,
                  dtype=np.float32).astype(np.float32)
    embT2B[D] = -bvec
    return np.ascontiguousarray(embT2B), bvec


def kernel(inputs: np.ndarray, emb_w: np.ndarray, _trace=False):
    inputs = np.ascontiguousarray(np.asarray(inputs, dtype=np.float32))
    emb_w = np.ascontiguousarray(np.asarray(emb_w, dtype=np.float32))
    assert inputs.shape == (N, D) and emb_w.shape == (K, D)

    nc = _get_nc()
    embT2B, bvec = _host_prep(emb_w)
    in_maps = [
        {"x": inputs[i * NS:(i + 1) * NS], "emb": emb_w, "embT2B": embT2B}
        for i in range(N_CORES)
    ]
    out = run_bass_kernel_spmd(nc, in_maps, core_ids=list(range(N_CORES)),
                               trace=_trace)
    res = out.results

    enc = np.concatenate([r["enc"] for r in res], axis=0)
    q = np.concatenate([r["q"] for r in res], axis=0)
    nt = NS // 128
    # device row r of core i lives at [p=r%128, t=r//128]
    idx = np.concatenate([r["idx"][:, 0::8].T.reshape(-1) for r in res])
    smax0 = np.concatenate([r["sm"][:, 0::8].T.reshape(-1) for r in res])
    smax1 = np.concatenate([r["sm"][:, 1::8].T.reshape(-1) for r in res])
    avals = np.concatenate([r["av"].T.reshape(-1) for r in res])

    # per-row min distance for the loss (fine-grained, unquantized)
    dmin = avals.astype(np.float64) - smax0.astype(np.float64)

    # ---- host repair of near-tie rows: reproduce the reference's f32
    # rounding (distances at magnitude ~64 quantize to a ~7.6e-6 grid)
    flagged = np.flatnonzero((smax0 - smax1) < GAP_THRESHOLD)
    if flagged.size:
        xr = inputs[flagged]
        a32 = np.sum(xr ** 2, axis=1, dtype=np.float32).astype(np.float32)
        m32 = xr @ emb_w.T
        d32 = ((a32[:, None] + bvec[None, :]).astype(np.float32)
               - np.float32(2.0) * m32).astype(np.float32)
        idx_fix = np.argmin(d32, axis=1)
        changed = idx_fix != idx[flagged]
        rows = flagged[changed]
        if rows.size:
            idx[rows] = idx_fix[changed]
            enc[rows] = 0.0
            enc[rows, idx[rows]] = 1.0
            q[rows] = emb_w[idx[rows]]
        dmin[flagged] = d32.min(axis=1).astype(np.float64)

    loss = np.float32((1.0 + COMMITMENT_COST) * (dmin.sum() / (N * D)))

    counts = np.bincount(idx, minlength=K).astype(np.float64)
    avg_probs = counts / N
    perplexity = np.float32(
        np.exp(-np.sum(avg_probs * np.log(avg_probs + 1e-10))))

    # straight-through output, f32 exact as reference computes it
    quantized_st = inputs + (q - inputs)

    if _trace:
        return (loss, quantized_st, perplexity, enc), out
    return loss, quantized_st, perplexity, enc
